# revision 1
# baseline (speedup 1.0000x reference)
"""Distributed FlashRotarySelfAttention kernel for 8 TRN2 NeuronCores.

Reference computation (per nn_FlashRotarySelfAttention):
  qkv = x @ Wqkv;  k, q, v = split(qkv, 3)  [k first!]
  k, q = rope(k), rope(q)
  out = causal_softmax(q k^T / sqrt(Dh)) @ v
  return out @ Wproj

Sharding: tensor-parallel over heads. Core i owns heads {2i, 2i+1}:
  - column-parallel Wqkv (k|q|v columns of its 2 heads)
  - attention fully local per (batch, head)
  - one AllGather per batch of the attention outputs (transposed,
    c-major); batch 0's gather overlaps batch 1's attention compute
  - column-parallel Wproj: each core computes 256 output channels
Host concatenates + transposes the per-core outputs.

All matmuls run in bf16 with fp32 PSUM accumulation. x is transposed
on-chip: f32 tile load -> DVE cast to bf16 -> XBAR SBUF->SBUF transpose
(no HBM roundtrip). Softmax skips the max-subtraction (scores are O(10)
here, exp is safe in fp32); the denominator is accumulated on DVE and
reduced across partitions by a single ones-matmul per group, which also
replicates it across partitions so normalization is an aligned multiply.
"""

from contextlib import ExitStack

import numpy as np
import ml_dtypes

import concourse.bacc as bacc
import concourse.mybir as mybir
import concourse.tile as tile
from concourse.bass_utils import run_bass_kernel_spmd

# Problem shapes (hardcoded per contest rules).
B, S, C, H = 2, 2048, 2048, 16
Dh = C // H                      # 128
BS = B * S                       # 4096
N_CORES = 8
H_LOC = H // N_CORES             # 2 heads per core
W_LOC = 3 * H_LOC * Dh           # 768 local qkv columns
CO_LOC = C // N_CORES            # 256 output channels per core
ROPE_THETA = 10000.0
SCALE = float(Dh) ** -0.5

F32 = mybir.dt.float32
BF16 = mybir.dt.bfloat16

P = 128            # partitions
QCH = 512          # q-chunk (matmul free dim)
N_SC = BS // QCH   # 8 s-chunks over B*S
N_CC = C // P      # 16 contraction chunks
N_QC = S // QCH    # 4 q-chunks per batch
N_KT = S // P      # 16 k-tiles per batch


def _host_constants():
    """Input-independent tables computed on host (compile-time constants)."""
    half = Dh // 2
    inv_freq = 1.0 / (ROPE_THETA ** (np.arange(0, half, dtype=np.float64) / half))
    ang = np.arange(S, dtype=np.float64)[None, :] * inv_freq[:, None]   # [64, S]
    cos_t = np.tile(np.cos(ang), (2, 1)).astype(ml_dtypes.bfloat16)
    sin_t = np.tile(np.sin(ang), (2, 1)).astype(ml_dtypes.bfloat16)
    # Causal 0/1 masks for diagonal score tiles, scoresT layout [k_local, q_local].
    # Tile j (k-tile index j within the q-chunk): keep iff q_local >= 128*j + k_local.
    kk = np.arange(P)[:, None]
    qq = np.arange(QCH)[None, :]
    masks = np.stack(
        [(qq >= P * j + kk) for j in range(4)], axis=0
    ).astype(ml_dtypes.bfloat16)                                        # [4, 128, 512]
    ident = np.eye(P, dtype=ml_dtypes.bfloat16)
    ones = np.ones((P, P), dtype=ml_dtypes.bfloat16)
    return cos_t, sin_t, masks, ident, ones


def build_nc():
    nc = bacc.Bacc(None, num_devices=N_CORES)

    x_in = nc.declare_dram_parameter("x", [BS, C], F32, isOutput=False)
    wqkv_in = nc.declare_dram_parameter("wqkv", [C, W_LOC], F32, isOutput=False)
    wproj_in = nc.declare_dram_parameter("wproj", [C, CO_LOC], F32, isOutput=False)
    cos_in = nc.declare_dram_parameter("cos_t", [Dh, S], BF16, isOutput=False)
    sin_in = nc.declare_dram_parameter("sin_t", [Dh, S], BF16, isOutput=False)
    masks_in = nc.declare_dram_parameter("masks", [4, P, QCH], BF16, isOutput=False)
    ident_in = nc.declare_dram_parameter("ident", [P, P], BF16, isOutput=False)
    ones_in = nc.declare_dram_parameter("ones", [P, P], BF16, isOutput=False)
    out_ext = nc.declare_dram_parameter("outT", [CO_LOC, BS], F32, isOutput=True)

    with tile.TileContext(nc) as tc, ExitStack() as ctx:
        consts = ctx.enter_context(tc.tile_pool(name="consts", bufs=1))
        qkvp = ctx.enter_context(tc.tile_pool(name="qkvp", bufs=1))
        xf_pool = ctx.enter_context(tc.tile_pool(name="xf", bufs=2))
        xb_pool = ctx.enter_context(tc.tile_pool(name="xb", bufs=2))
        xt_pool = ctx.enter_context(tc.tile_pool(name="xt", bufs=2))
        rope_pool = ctx.enter_context(tc.tile_pool(name="rope", bufs=4))
        probs_pool = ctx.enter_context(tc.tile_pool(name="probs", bufs=4))
        attn_pool = ctx.enter_context(tc.tile_pool(name="attn", bufs=2))
        accp_pool = ctx.enter_context(tc.tile_pool(name="accp", bufs=4))
        gt_pool = ctx.enter_context(tc.tile_pool(name="gt", bufs=2))
        outp_pool = ctx.enter_context(tc.tile_pool(name="outp", bufs=1))
        dram = ctx.enter_context(tc.tile_pool(name="dram", bufs=1, space="DRAM"))
        mmps = ctx.enter_context(tc.tile_pool(name="mmps", bufs=2, space="PSUM"))
        sps_pool = ctx.enter_context(tc.tile_pool(name="sps", bufs=4, space="PSUM"))
        ops_pool = ctx.enter_context(tc.tile_pool(name="ops", bufs=2, space="PSUM"))

        # ---- Startup: first wqkv slice so the PE can start ASAP ----------
        wqkv_sb = consts.tile([P, N_CC, W_LOC], BF16)
        wqkv_src = wqkv_in.rearrange("(o p) w -> p o w", p=P)
        nc.gpsimd.dma_start(wqkv_sb[:, 0:4, :], wqkv_src[:, 0:4, :])
        nc.gpsimd.dma_start(wqkv_sb[:, 4:16, :], wqkv_src[:, 4:16, :])

        cos_sb = consts.tile([Dh, S], BF16)
        nc.scalar.dma_start(cos_sb[:], cos_in[:])
        sin_sb = consts.tile([Dh, S], BF16)
        nc.scalar.dma_start(sin_sb[:], sin_in[:])
        masks_sb = consts.tile([P, 4, QCH], BF16)
        nc.scalar.dma_start(masks_sb[:], masks_in.rearrange("j p q -> p j q"))
        ones_sb = consts.tile([P, P], BF16)
        nc.scalar.dma_start(ones_sb[:], ones_in[:])
        ident_sb = consts.tile([P, P], BF16)
        nc.scalar.dma_start(ident_sb[:], ident_in[:])

        wproj_sb = consts.tile([P, N_CC, CO_LOC], BF16)
        nc.gpsimd.dma_start(wproj_sb[:], wproj_in.rearrange("(o p) w -> p o w", p=P))

        # Resident activations: d-major q/k, k-major v. bh = h_local*2 + b
        q_sb = qkvp.tile([P, 2 * H_LOC, S], BF16)
        k_sb = qkvp.tile([P, 2 * H_LOC, S], BF16)
        v_sb = qkvp.tile([P, B, N_KT, H_LOC * Dh], BF16)

        # ---- QKV: x load/cast/transpose on-chip, matmuls, RoPE -----------
        def x_prep(sc):
            # build x^T tile [c_in(128, o), s(512)]: per 128-row slice,
            # f32 load -> bf16 cast -> XBAR SBUF->SBUF transpose
            g0 = sc * QCH
            xt = xt_pool.tile([P, N_CC, QCH], BF16, tag="xt", name=f"xt{sc}")
            for blk in range(QCH // P):
                r0 = g0 + blk * P
                xf = xf_pool.tile([P, C], F32, tag="xf")
                nc.scalar.dma_start(xf[:], x_in[r0:r0 + P, :])
                xb = xb_pool.tile([P, C], BF16, tag="xb")
                nc.vector.tensor_copy(xb[:], xf[:])
                nc.sync.dma_start_transpose(
                    xt[:, :, blk * P:(blk + 1) * P], xb[:]
                )
            return xt

        def qkv_chunk(sc, xt, xt_next):
            g0 = sc * QCH
            b = g0 // S
            s0 = g0 - b * S              # position offset within batch
            cos_c = cos_sb[:, s0:s0 + QCH]
            sin_c = sin_sb[:, s0:s0 + QCH]

            # v: computed directly in k-major [s_tile, 2 heads * Dh]
            for blk in range(QCH // P):
                st = s0 // P + blk
                pv = sps_pool.tile([P, QCH], F32, tag="sc")
                for cc in range(N_CC):
                    nc.tensor.matmul(
                        pv[:, :H_LOC * Dh],
                        lhsT=xt[:, cc, blk * P:(blk + 1) * P],
                        rhs=wqkv_sb[:, cc, 4 * P:],
                        start=(cc == 0),
                        stop=(cc == N_CC - 1),
                    )
                nc.vector.tensor_copy(v_sb[:, b, st, :], pv[:, :H_LOC * Dh])

            for ct in range(4):
                # k (ct 0,1) and q (ct 2,3): RoPE -> bf16 resident
                ps = mmps.tile([P, QCH], F32, tag="mm")
                for cc in range(N_CC):
                    nc.tensor.matmul(
                        ps[:],
                        lhsT=wqkv_sb[:, cc, ct * P:(ct + 1) * P],
                        rhs=xt[:, cc, :],
                        start=(cc == 0),
                        stop=(cc == N_CC - 1),
                    )
                hl = ct % 2
                dst = k_sb if ct < 2 else q_sb
                bh = hl * 2 + b
                lo = ps[0:64, :]
                hi = ps[64:128, :]
                t1 = rope_pool.tile([64, QCH], BF16, tag="rt")
                t2 = rope_pool.tile([64, QCH], BF16, tag="rt")
                t3 = rope_pool.tile([64, QCH], BF16, tag="rt")
                t4 = rope_pool.tile([64, QCH], BF16, tag="rt")
                nc.vector.tensor_tensor(t1[:], lo, cos_c[0:64, :],
                                        mybir.AluOpType.mult)
                nc.vector.tensor_tensor(t2[:], hi, sin_c[64:128, :],
                                        mybir.AluOpType.mult)
                nc.vector.tensor_tensor(
                    dst[0:64, bh, s0:s0 + QCH],
                    t1[:], t2[:], mybir.AluOpType.subtract,
                )
                nc.vector.tensor_tensor(t3[:], hi, cos_c[64:128, :],
                                        mybir.AluOpType.mult)
                nc.vector.tensor_tensor(t4[:], lo, sin_c[0:64, :],
                                        mybir.AluOpType.mult)
                nc.vector.tensor_tensor(
                    dst[64:128, bh, s0:s0 + QCH],
                    t3[:], t4[:], mybir.AluOpType.add,
                )

        # ---- Phase 3: attention; per-batch AllGather + projection --------
        ag_in = [dram.tile([H_LOC * Dh, S], BF16, name=f"agi{j}")
                 for j in range(B)]
        ag_out = [dram.tile([C, S], BF16, name=f"ago{j}") for j in range(B)]

        def attn_group(b, qc, hl):
                    n_kt = (QCH // P) * (qc + 1)
                    bh = hl * 2 + b
                    po = ops_pool.tile([P, QCH], F32, tag="po")
                    acc = accp_pool.tile([P, QCH], BF16, tag="acc")
                    for kt in range(n_kt):
                        jj = kt - (QCH // P) * qc
                        # diagonal tiles: columns below 128*jj are fully
                        # masked -- skip computing them entirely
                        off = P * jj if jj > 0 else 0
                        pscore = sps_pool.tile([P, QCH], F32, tag="sc")
                        nc.tensor.matmul(
                            pscore[:, off:],
                            lhsT=k_sb[:, bh, kt * P:(kt + 1) * P],
                            rhs=q_sb[:, bh, qc * QCH + off:(qc + 1) * QCH],
                            start=True, stop=True,
                        )
                        pr = probs_pool.tile([P, QCH], BF16, tag="pr")
                        nc.scalar.activation(
                            pr[:, off:], pscore[:, off:],
                            mybir.ActivationFunctionType.Exp,
                            scale=SCALE,
                        )
                        if jj >= 0:
                            nc.vector.tensor_tensor(
                                pr[:, off:], pr[:, off:],
                                masks_sb[:, jj, off:],
                                mybir.AluOpType.mult,
                            )
                        if kt == 0:
                            nc.vector.tensor_copy(acc[:], pr[:])
                        else:
                            nc.vector.tensor_tensor(
                                acc[:, off:], acc[:, off:], pr[:, off:],
                                mybir.AluOpType.add,
                            )
                        nc.tensor.matmul(
                            po[:, off:], lhsT=v_sb[:, b, kt, hl * Dh:(hl + 1) * Dh],
                            rhs=pr[:, off:],
                            start=(kt == 0), stop=(kt == n_kt - 1),
                        )
                    pd = mmps.tile([P, QCH], F32, tag="mm")
                    nc.tensor.matmul(
                        pd[:], lhsT=ones_sb[:], rhs=acc[:],
                        start=True, stop=True,
                    )
                    recip = attn_pool.tile([P, QCH], F32, tag="rec")
                    nc.vector.reciprocal(recip[:], pd[:])
                    at = attn_pool.tile([P, QCH], BF16, tag="at")
                    nc.vector.tensor_tensor(
                        at[:], po[:], recip[:], mybir.AluOpType.mult
                    )
                    nc.scalar.dma_start(
                        ag_in[b][hl * Dh:(hl + 1) * Dh,
                                 qc * QCH:(qc + 1) * QCH],
                        at[:],
                    )

        def allgather(b):
            nc.gpsimd.collective_compute(
                "AllGather",
                mybir.AluOpType.bypass,
                replica_groups=[list(range(N_CORES))],
                ins=[ag_in[b][:].opt()],
                outs=[ag_out[b][:].opt()],
            )

        def projection(b):
            for qc in range(N_QC):
                gt = gt_pool.tile([P, N_CC, QCH], BF16, tag="gt")
                nc.scalar.dma_start(
                    gt[:],
                    ag_out[b][:, qc * QCH:(qc + 1) * QCH].rearrange(
                        "(o p) q -> p o q", p=P
                    ),
                )
                for ct in range(CO_LOC // P):
                    ps = mmps.tile([P, QCH], F32, tag="mm")
                    for cc in range(N_CC):
                        nc.tensor.matmul(
                            ps[:],
                            lhsT=wproj_sb[:, cc, ct * P:(ct + 1) * P],
                            rhs=gt[:, cc, :],
                            start=(cc == 0),
                            stop=(cc == N_CC - 1),
                        )
                    ot = outp_pool.tile([P, QCH], F32, tag="ot")
                    nc.vector.tensor_copy(ot[:], ps[:])
                    nc.scalar.dma_start(
                        out_ext[ct * P:(ct + 1) * P,
                                b * S + qc * QCH:b * S + (qc + 1) * QCH],
                        ot[:],
                    )

        xts = [None] * N_SC
        xts[0] = x_prep(0)
        for sc in range(N_SC):
            if sc + 1 < N_SC:
                xts[sc + 1] = x_prep(sc + 1)
            qkv_chunk(sc, xts[sc], None)
            xts[sc] = None
        for qc in range(N_QC):
            attn_group(0, qc, 0)
            attn_group(0, qc, 1)
        allgather(0)
        for qc in range(N_QC):
            attn_group(1, qc, 0)
            attn_group(1, qc, 1)
        allgather(1)
        projection(0)
        projection(1)

    nc.finalize()
    return nc


_NC_CACHE = None


def _get_nc():
    global _NC_CACHE
    if _NC_CACHE is None:
        _NC_CACHE = build_nc()
    return _NC_CACHE


def make_in_maps(x, Wqkv, Wproj):
    """Shard the full inputs across the 8 cores (host side)."""
    x2 = np.ascontiguousarray(np.asarray(x, dtype=np.float32).reshape(BS, C))
    Wqkv = np.asarray(Wqkv, dtype=np.float32)
    Wproj = np.asarray(Wproj, dtype=np.float32)
    cos_t, sin_t, masks, ident, ones = _host_constants()
    in_maps = []
    for i in range(N_CORES):
        h0 = H_LOC * i
        cols = []
        for part in range(3):  # k, q, v blocks (k first per reference)
            base = part * C + h0 * Dh
            cols.append(Wqkv[:, base:base + H_LOC * Dh])
        wqkv_loc = np.ascontiguousarray(np.concatenate(cols, axis=1))
        wproj_loc = np.ascontiguousarray(Wproj[:, i * CO_LOC:(i + 1) * CO_LOC])
        in_maps.append({
            "x": x2,
            "wqkv": wqkv_loc,
            "wproj": wproj_loc,
            "cos_t": cos_t,
            "sin_t": sin_t,
            "masks": masks,
            "ident": ident,
            "ones": ones,
        })
    return in_maps


def assemble_output(results):
    outT = np.concatenate([results[i]["outT"] for i in range(N_CORES)], axis=0)
    return np.ascontiguousarray(outT.T).reshape(B, S, C).astype(np.float32)


def kernel(x, Wqkv, Wproj):
    nc = _get_nc()
    in_maps = make_in_maps(x, Wqkv, Wproj)
    res = run_bass_kernel_spmd(nc, in_maps, core_ids=list(range(N_CORES)))
    return assemble_output(res.results)



# revision 2
# speedup vs baseline: 1.4726x; 1.4726x over previous
"""Distributed FlashRotarySelfAttention kernel for 8 TRN2 NeuronCores.

Reference computation (per nn_FlashRotarySelfAttention):
  qkv = x @ Wqkv;  k, q, v = split(qkv, 3)  [k first!]
  k, q = rope(k), rope(q)
  out = causal_softmax(q k^T / sqrt(Dh)) @ v
  return out @ Wproj

Sharding: tensor-parallel over heads for QKV+attention, position-parallel
for the projection. Core i owns heads {2i, 2i+1}:
  - column-parallel Wqkv (k|q|v columns of its 2 heads)
  - attention fully local per (batch, head)
  - one 8-core AllToAll per batch redistributes attention outputs from
    head-sharding to position-sharding (1 MB in -> 1 MB out per core;
    batch 0's AllToAll overlaps batch 1's attention compute)
  - projection: each core multiplies its 256 positions by the FULL Wproj
    producing s-major output rows directly (no host transpose)

All inputs are pre-cast/pre-transposed to bf16 on the host: x arrives
c-major tiled [128, 16, 4096] so QKV needs no on-chip cast or transpose.
Matmuls run bf16 with fp32 PSUM accumulation.

Softmax: causal masking is an additive -30000 bias written into the score
PSUM by an identity-weight matmul (start=True) before the score matmul
accumulates onto it; exp runs on the Scalar engine; the denominator is
accumulated across k-tiles by an all-ones matmul into a second PSUM bank
(broadcast across partitions); normalization is reciprocal_approx_fast +
one multiply on DVE. RoPE uses sign-folded cos/sin tables: 2 half-width +
2 full-width DVE ops per 128-channel group.
"""

from contextlib import ExitStack

import numpy as np
import ml_dtypes

import concourse.bacc as bacc
import concourse.mybir as mybir
import concourse.tile as tile
from concourse.bass_utils import run_bass_kernel_spmd

# Problem shapes (hardcoded per contest rules).
B, S, C, H = 2, 2048, 2048, 16
Dh = C // H                      # 128
BS = B * S                       # 4096
N_CORES = 8
H_LOC = H // N_CORES             # 2 heads per core
W_LOC = 3 * H_LOC * Dh           # 768 local qkv columns
POS_LOC = S // N_CORES           # 256 positions per core per batch
ROPE_THETA = 10000.0
SCALE = float(Dh) ** -0.5
NEG_BIAS = -30000.0              # additive causal-mask bias (pre-scale)

F32 = mybir.dt.float32
BF16 = mybir.dt.bfloat16

P = 128            # partitions
QCH = 512          # q-chunk (matmul free dim)
N_CC = C // P      # 16 contraction chunks
N_QC = S // QCH    # 4 q-chunks per batch
N_KT = S // P      # 16 k-tiles per batch


def _host_constants():
    """Input-independent tables computed on host (compile-time constants)."""
    half = Dh // 2
    inv_freq = 1.0 / (ROPE_THETA ** (np.arange(0, half, dtype=np.float64) / half))
    ang = np.arange(S, dtype=np.float64)[None, :] * inv_freq[:, None]   # [64, S]
    cos = np.cos(ang)
    sin = np.sin(ang)
    # Full-width rope tables: out = t*cosf + swap_halves(t)*sinn
    cosf = np.concatenate([cos, cos], axis=0).astype(ml_dtypes.bfloat16)
    sinn = np.concatenate([-sin, sin], axis=0).astype(ml_dtypes.bfloat16)
    # Causal bias triangle for the diagonal 128x128 block of each score
    # tile: keep (bias 0) iff q_local >= k_local, else NEG_BIAS.
    kk = np.arange(P)[:, None]
    qq = np.arange(P)[None, :]
    tri = np.where(qq >= kk, 0.0, NEG_BIAS).astype(ml_dtypes.bfloat16)
    ident = np.eye(P, dtype=ml_dtypes.bfloat16)
    ones = np.ones((P, P), dtype=ml_dtypes.bfloat16)
    return cosf, sinn, tri, ident, ones


def _tile_cmajor(a):
    """[C, N] -> [128, C//128, N] with channel c -> (c % 128, c // 128)."""
    cdim, n = a.shape
    return np.ascontiguousarray(
        a.reshape(cdim // P, P, n).transpose(1, 0, 2)
    )


def build_nc():
    nc = bacc.Bacc(None, num_devices=N_CORES)

    xt_in = nc.declare_dram_parameter("xt", [P, N_CC, BS], BF16, isOutput=False)
    wqkv_in = nc.declare_dram_parameter("wqkv", [P, N_CC, W_LOC], BF16, isOutput=False)
    wproj_in = nc.declare_dram_parameter("wproj", [P, N_CC, C], BF16, isOutput=False)
    cosf_in = nc.declare_dram_parameter("cosf", [Dh, S], BF16, isOutput=False)
    sinn_in = nc.declare_dram_parameter("sinn", [Dh, S], BF16, isOutput=False)
    tri_in = nc.declare_dram_parameter("tri", [P, P], BF16, isOutput=False)
    ident_in = nc.declare_dram_parameter("ident", [P, P], BF16, isOutput=False)
    ones_in = nc.declare_dram_parameter("ones", [P, P], BF16, isOutput=False)
    out_ext = nc.declare_dram_parameter("out", [B * POS_LOC, C], F32, isOutput=True)

    with tile.TileContext(nc) as tc, ExitStack() as ctx:
        consts = ctx.enter_context(tc.tile_pool(name="consts", bufs=1))
        qkvp = ctx.enter_context(tc.tile_pool(name="qkvp", bufs=1))
        xt_pool = ctx.enter_context(tc.tile_pool(name="xt", bufs=2))
        rope_pool = ctx.enter_context(tc.tile_pool(name="rope", bufs=4))
        probs_pool = ctx.enter_context(tc.tile_pool(name="probs", bufs=4))
        rec_pool = ctx.enter_context(tc.tile_pool(name="rec", bufs=2))
        at_pool = ctx.enter_context(tc.tile_pool(name="at", bufs=2))
        gt_pool = ctx.enter_context(tc.tile_pool(name="gt", bufs=2))
        wp_pool = ctx.enter_context(tc.tile_pool(name="wp", bufs=2))
        outp_pool = ctx.enter_context(tc.tile_pool(name="outp", bufs=2))
        dram = ctx.enter_context(tc.tile_pool(name="dram", bufs=1, space="DRAM"))
        mmps = ctx.enter_context(tc.tile_pool(name="mmps", bufs=2, space="PSUM"))
        sps_pool = ctx.enter_context(tc.tile_pool(name="sps", bufs=2, space="PSUM"))
        avp_pool = ctx.enter_context(tc.tile_pool(name="avp", bufs=2, space="PSUM"))
        dnp_pool = ctx.enter_context(tc.tile_pool(name="dnp", bufs=2, space="PSUM"))

        # ---- Startup: weights + tables ----------------------------------
        wqkv_sb = consts.tile([P, N_CC, W_LOC], BF16)
        nc.sync.dma_start(wqkv_sb[:, 0:4, :], wqkv_in[:, 0:4, :])
        nc.sync.dma_start(wqkv_sb[:, 4:16, :], wqkv_in[:, 4:16, :])

        cosf_sb = consts.tile([Dh, S], BF16)
        nc.sync.dma_start(cosf_sb[:], cosf_in[:])
        sinn_sb = consts.tile([Dh, S], BF16)
        nc.sync.dma_start(sinn_sb[:], sinn_in[:])
        tri_sb = consts.tile([P, P], BF16)
        nc.sync.dma_start(tri_sb[:], tri_in[:])
        ident_sb = consts.tile([P, P], BF16)
        nc.sync.dma_start(ident_sb[:], ident_in[:])
        ones_sb = consts.tile([P, P], BF16)
        nc.sync.dma_start(ones_sb[:], ones_in[:])

        # Resident activations: d-major q/k (dim1 = hl*2 + b), k-major v.
        q_sb = qkvp.tile([P, 2 * H_LOC, S], BF16)
        k_sb = qkvp.tile([P, 2 * H_LOC, S], BF16)
        v_sb = qkvp.tile([P, B, N_KT, H_LOC * Dh], BF16)

        # ---- QKV: straight bf16 loads, matmuls, full-width RoPE ---------
        def qkv_chunk(sc):
            g0 = sc * QCH
            b = g0 // S
            s0 = g0 - b * S              # position offset within batch
            xt = xt_pool.tile([P, N_CC, QCH], BF16, tag="xt", name=f"xt{sc}")
            nc.sync.dma_start(xt[:], xt_in[:, :, g0:g0 + QCH])

            # v: s-major [pos_tile, 2 heads * Dh]
            for blk in range(QCH // P):
                st = s0 // P + blk
                pv = sps_pool.tile([P, QCH], F32, tag="sc")
                for cc in range(N_CC):
                    nc.tensor.matmul(
                        pv[:, :H_LOC * Dh],
                        lhsT=xt[:, cc, blk * P:(blk + 1) * P],
                        rhs=wqkv_sb[:, cc, 4 * P:],
                        start=(cc == 0),
                        stop=(cc == N_CC - 1),
                    )
                nc.scalar.copy(v_sb[:, b, st, :], pv[:, :H_LOC * Dh])

            # k (ct 0,1) and q (ct 2,3): d-major matmul + RoPE
            cos_c = cosf_sb[:, s0:s0 + QCH]
            sin_c = sinn_sb[:, s0:s0 + QCH]
            for ct in range(4):
                ps = mmps.tile([P, QCH], F32, tag="mm")
                for cc in range(N_CC):
                    nc.tensor.matmul(
                        ps[:],
                        lhsT=wqkv_sb[:, cc, ct * P:(ct + 1) * P],
                        rhs=xt[:, cc, :],
                        start=(cc == 0),
                        stop=(cc == N_CC - 1),
                    )
                hl = ct % 2
                dst = k_sb if ct < 2 else q_sb
                bh = hl * 2 + b
                # out = ps*cosf + swap_halves(ps)*sinn  (sign folded in sinn)
                tmp = rope_pool.tile([P, QCH], BF16, tag="rt")
                c1 = rope_pool.tile([P, QCH], BF16, tag="rt")
                nc.vector.tensor_tensor(tmp[0:64, :], ps[64:128, :],
                                        sin_c[0:64, :], mybir.AluOpType.mult)
                nc.vector.tensor_tensor(tmp[64:128, :], ps[0:64, :],
                                        sin_c[64:128, :], mybir.AluOpType.mult)
                nc.vector.tensor_tensor(c1[:], ps[:], cos_c,
                                        mybir.AluOpType.mult)
                nc.vector.tensor_tensor(
                    dst[:, bh, s0:s0 + QCH], c1[:], tmp[:],
                    mybir.AluOpType.add,
                )

        # ---- Attention + per-batch AllToAll + projection ----------------
        a2a_in = [dram.tile([C, POS_LOC], BF16, name=f"a2i{j}")
                  for j in range(B)]
        a2a_out = [dram.tile([C, POS_LOC], BF16, name=f"a2o{j}")
                   for j in range(B)]

        def attn_group(b, qc, hl):
            n_kt = (QCH // P) * (qc + 1)
            bh = hl * 2 + b
            po = avp_pool.tile([P, QCH], F32, tag="po")
            pd = dnp_pool.tile([P, QCH], F32, tag="pd")
            for kt in range(n_kt):
                jj = kt - (QCH // P) * qc
                # diagonal tiles: columns below 128*jj are fully masked --
                # skip them; the 128-wide triangle gets an additive bias
                off = P * jj if jj > 0 else 0
                pscore = sps_pool.tile([P, QCH], F32, tag="sc")
                if jj >= 0:
                    nc.tensor.matmul(
                        pscore[:, P * jj:P * (jj + 1)],
                        lhsT=ident_sb[:], rhs=tri_sb[:],
                        start=True, stop=False,
                    )
                nc.tensor.matmul(
                    pscore[:, off:],
                    lhsT=k_sb[:, bh, kt * P:(kt + 1) * P],
                    rhs=q_sb[:, bh, qc * QCH + off:(qc + 1) * QCH],
                    start=(jj < 0), stop=True,
                )
                pr = probs_pool.tile([P, QCH], BF16, tag="pr")
                nc.scalar.activation(
                    pr[:, off:], pscore[:, off:],
                    mybir.ActivationFunctionType.Exp,
                    scale=SCALE,
                )
                nc.tensor.matmul(
                    po[:, off:],
                    lhsT=v_sb[:, b, kt, hl * Dh:(hl + 1) * Dh],
                    rhs=pr[:, off:],
                    start=(kt == 0), stop=(kt == n_kt - 1),
                )
                nc.tensor.matmul(
                    pd[:, off:], lhsT=ones_sb[:], rhs=pr[:, off:],
                    start=(kt == 0), stop=(kt == n_kt - 1),
                )
            rec = rec_pool.tile([P, QCH], F32, tag="rec")
            nc.vector.reciprocal_approx_fast(rec[:], pd[:])
            at = at_pool.tile([P, QCH], BF16, tag="at")
            nc.vector.tensor_tensor(
                at[:], po[:], rec[:], mybir.AluOpType.mult
            )
            # scatter the two 256-position halves into the AllToAll input:
            # row layout of a2a_in[b]: blk*256 + hl*128 + hd
            for half in range(2):
                blk = 2 * qc + half
                r0 = blk * 2 * P + hl * P
                nc.sync.dma_start(
                    a2a_in[b][r0:r0 + P, :],
                    at[:, half * POS_LOC:(half + 1) * POS_LOC],
                )

        def alltoall(b):
            nc.gpsimd.collective_compute(
                "AllToAll",
                mybir.AluOpType.bypass,
                replica_groups=[list(range(N_CORES))],
                ins=[a2a_in[b][:].opt()],
                outs=[a2a_out[b][:].opt()],
            )

        def projection(b):
            gt = gt_pool.tile([P, N_CC, POS_LOC], BF16, tag="gt")
            nc.sync.dma_start(
                gt[:], a2a_out[b].rearrange("(o p) q -> p o q", p=P)
            )
            for oc in range(C // QCH):
                wp = wp_pool.tile([P, N_CC, QCH], BF16, tag="wp")
                nc.sync.dma_start(
                    wp[:], wproj_in[:, :, oc * QCH:(oc + 1) * QCH]
                )
                for pt in range(POS_LOC // P):
                    ps = mmps.tile([P, QCH], F32, tag="mm")
                    for cc in range(N_CC):
                        nc.tensor.matmul(
                            ps[:],
                            lhsT=gt[:, cc, pt * P:(pt + 1) * P],
                            rhs=wp[:, cc, :],
                            start=(cc == 0),
                            stop=(cc == N_CC - 1),
                        )
                    ot = outp_pool.tile([P, QCH], F32, tag="ot")
                    nc.scalar.copy(ot[:], ps[:])
                    nc.sync.dma_start(
                        out_ext[b * POS_LOC + pt * P:b * POS_LOC + (pt + 1) * P,
                                oc * QCH:(oc + 1) * QCH],
                        ot[:],
                    )

        for sc in range(4):
            qkv_chunk(sc)
        for qc in range(N_QC):
            attn_group(0, qc, 0)
            attn_group(0, qc, 1)
        alltoall(0)
        for sc in range(4, 8):
            qkv_chunk(sc)
        for qc in range(N_QC):
            attn_group(1, qc, 0)
            attn_group(1, qc, 1)
        alltoall(1)
        projection(0)
        projection(1)

    nc.finalize()
    return nc


_NC_CACHE = None


def _get_nc():
    global _NC_CACHE
    if _NC_CACHE is None:
        _NC_CACHE = build_nc()
    return _NC_CACHE


def make_in_maps(x, Wqkv, Wproj):
    """Shard + pre-transpose the full inputs across the 8 cores (host)."""
    x2 = np.asarray(x, dtype=np.float32).reshape(BS, C)
    xt = _tile_cmajor(x2.T.astype(ml_dtypes.bfloat16))        # [128,16,4096]
    Wqkv = np.asarray(Wqkv, dtype=np.float32)
    Wproj = np.asarray(Wproj, dtype=np.float32)
    wproj_t = _tile_cmajor(Wproj.astype(ml_dtypes.bfloat16))  # [128,16,2048]
    cosf, sinn, tri, ident, ones = _host_constants()
    in_maps = []
    for i in range(N_CORES):
        h0 = H_LOC * i
        cols = []
        for part in range(3):  # k, q, v blocks (k first per reference)
            base = part * C + h0 * Dh
            cols.append(Wqkv[:, base:base + H_LOC * Dh])
        wqkv_loc = _tile_cmajor(
            np.concatenate(cols, axis=1).astype(ml_dtypes.bfloat16)
        )
        in_maps.append({
            "xt": xt,
            "wqkv": wqkv_loc,
            "wproj": wproj_t,
            "cosf": cosf,
            "sinn": sinn,
            "tri": tri,
            "ident": ident,
            "ones": ones,
        })
    return in_maps


def assemble_output(results):
    out = np.empty((B, S, C), dtype=np.float32)
    for i in range(N_CORES):
        o = results[i]["out"].reshape(B, POS_LOC, C)
        out[:, i * POS_LOC:(i + 1) * POS_LOC, :] = o
    return out


def kernel(x, Wqkv, Wproj):
    nc = _get_nc()
    in_maps = make_in_maps(x, Wqkv, Wproj)
    res = run_bass_kernel_spmd(nc, in_maps, core_ids=list(range(N_CORES)))
    return assemble_output(res.results)
